# revision 8
# baseline (speedup 1.0000x reference)
"""Sparse attention (template/search) Trainium2 kernel.

Model (per batch b):
  qkv = x @ qkv_w.T                  -> split to q, k, v heads (12 heads, hd=64)
  template tokens   [0, 256)  attend to template keys only
  search   tokens [256, 1280) attend to all 1280 keys
  out = softmax(q k^T / 8) v   per head, concat heads, @ proj_w.T + proj_b

Sharding: data-parallel over batch, one batch per NeuronCore (8 cores).
No collectives needed.

Layout strategy per core (v3):
  - x / qkv_w / proj_w are PE-transposed (fp32 transpose mode, then the
    PSUM->SBUF copy casts to bf16) to xT [C, NTOK], wT [C, 3C], pwT [C, C].
  - startup is DMA-critical, so only the minimum input set is loaded and
    transposed before attention starts: x, the q0/k0 weight rows, and the
    v weights.  The q/k weight rows for pairs 1-5 stream in later and are
    transposed inside the attention loop as filler work.
  - q,k computed feature-major into a 2-slot rotating buffer (slot =
    pair%2): qk[P, slot, {q,k}, NTOK] (q pre-scaled by 1/8).
  - v computed token-major, augmented per head as [1 | 63 zeros | v]:
    row 0 of the AV output is the softmax denominator.
  - scores computed TRANSPOSED: S.T[tk, tq] = K_h @ Q_h.T; the two heads
    of a pair fill the two halves of one [128, 1024] PSUM tile -> ONE exp
    instruction per (pair, cj, tk) covers both heads (N=1024 amortizes
    the ~300-cycle ACT instruction overhead).
  - search loop is cj-outer (two 512-token query chunks) so each head's
    AV accumulator is one PSUM bank: banks = 4 (scores, double-buffered)
    + 2 (AV accumulators) + 2 (fillers/transposes) = 8.
  - normalize fully off the ACT queue: DVE copy PSUM->SBUF, gpsimd
    partition_broadcast of row 0, DVE approx reciprocal, DVE multiply.
  - pair 0's template block is hoisted into the startup stream (its
    scores need only the first q/k chunk) so the ACT exp pipeline starts
    ~15us in; pair 5 runs its template LAST so the cj-1 projection tiles
    (t6-t9) overlap it, and tiles t2-t5 overlap cj 1 itself.
  - output DMAs are split in row halves across two queues.

Scheduling: attention paces ACT(exp) and PE about evenly; all qkv / v /
deferred-weight-transpose / proj work is software-pipelined into the
search loops as filler.  All matmuls bf16 (fp32 PSUM accumulation).
"""

import numpy as np

import concourse.bacc as bacc
import concourse.mybir as mybir
import concourse.tile as tile
from concourse.masks import make_identity

P = 128
NTOK = 1280
C = 768
H = 12
HD = 64
NT = 256          # template tokens  [0, NT)
TT = NTOK // P    # 10 token tiles
CT = C // P       # 6 channel tiles
SCALE = HD ** -0.5

F32 = mybir.dt.float32
BF16 = mybir.dt.bfloat16
EXP = mybir.ActivationFunctionType.Exp
MULT = mybir.AluOpType.mult
ADD = mybir.AluOpType.add


def build_nc():
    from contextlib import ExitStack

    nc = bacc.Bacc("TRN2", target_bir_lowering=False, debug=False, num_devices=8)
    x_ext = nc.dram_tensor("x", [NTOK, C], F32, kind="ExternalInput")
    w_ext = nc.dram_tensor("qkv_w", [3 * C, C], F32, kind="ExternalInput")
    pw_ext = nc.dram_tensor("proj_w", [C, C], F32, kind="ExternalInput")
    pb_ext = nc.dram_tensor("proj_b", [1, C], F32, kind="ExternalInput")
    out_ext = nc.dram_tensor("out", [NTOK, C], F32, kind="ExternalOutput")

    with tile.TileContext(nc) as tc, ExitStack() as ctx:
        const = ctx.enter_context(tc.tile_pool(name="const", bufs=1))
        big = ctx.enter_context(tc.tile_pool(name="big", bufs=1))
        # PSUM budget (16KB/partition): sc 2x4KB + ot 2x2KB + fill 2x2KB
        ps_sc = ctx.enter_context(tc.tile_pool(name="ps_sc", bufs=2, space="PSUM"))
        ps_ot = ctx.enter_context(tc.tile_pool(name="ps_ot", bufs=2, space="PSUM"))
        ps_fill = ctx.enter_context(tc.tile_pool(name="ps_fill", bufs=2, space="PSUM"))
        pts = ctx.enter_context(tc.tile_pool(name="pts", bufs=3))
        dn = ctx.enter_context(tc.tile_pool(name="dn", bufs=2))
        rbp = ctx.enter_context(tc.tile_pool(name="rbp", bufs=2))
        outp = ctx.enter_context(tc.tile_pool(name="outp", bufs=3))

        ident = const.tile([P, P], F32)
        make_identity(nc, ident)
        # HAM warmup: keep the PE busy during the initial input-DMA wait so
        # its clock gate opens (1.2 -> 2.4 GHz) before the real transpose
        # and qkv stream begins.
        warm_ps = ps_fill.tile([P, 512], F32, tag="fill")
        for i in range(24):
            nc.tensor.transpose(warm_ps[:, :P], ident[:], ident[:])
        nc.vector.tensor_copy(ident[:], warm_ps[:, :P])
        bias_bc = const.tile([P, C], F32)
        bias_row = const.tile([1, C], F32)
        nc.sync.dma_start(bias_row[:], pb_ext.ap())
        nc.gpsimd.partition_broadcast(bias_bc[:], bias_row[0:1, :])

        xT = big.tile([P, CT, NTOK], BF16)     # x.T  (feature-major x)
        wT = big.tile([P, CT, 3 * C], BF16)    # qkv_w.T
        pwT = big.tile([P, CT, C], BF16)       # proj_w.T
        pg = big.tile([P, CT, C], F32)         # proj_w fp32 (transposed pair 4)
        # deferred q/k weight rows j1-j5: rotating staging, DMA'd one
        # pair ahead of their transpose (a pure-DMA filler)
        wdefp = ctx.enter_context(tc.tile_pool(name='wdefp', bufs=3))
        wdef_tiles = {}

        def emit_wd_dma(j):
            t = wdefp.tile([P, 2, C], F32, tag='wd', name=f'wd{j}')
            wdef_tiles[j] = t
            for g in range(2):
                nc.sync.dma_start(t[:, g, :],
                                  w_ext.ap()[(g * 6 + j) * P:
                                             (g * 6 + j + 1) * P, :])

        def transpose_to(srcs, dsts):
            """PE-transpose fp32 [128,128] blocks through the filler PSUM
            ring; dsts are per-block [128,128] bf16 APs (copied individually
            when non-contiguous) or a single contiguous AP."""
            i = 0
            while i < len(srcs):
                n = min(4, len(srcs) - i)
                pt = ps_fill.tile([P, 512], F32, tag="fill")
                for j in range(n):
                    nc.tensor.transpose(
                        pt[:, j * P:(j + 1) * P], srcs[i + j], ident[:]
                    )
                for j in range(n):
                    nc.vector.tensor_copy(dsts[i + j], pt[:, j * P:(j + 1) * P])
                i += n

        def transpose_blocks(srcs, dst_full):
            """Contiguous-destination variant: one DVE copy per group."""
            i = 0
            while i < len(srcs):
                n = min(4, len(srcs) - i)
                pt = ps_fill.tile([P, 512], F32, tag="fill")
                for j in range(n):
                    nc.tensor.transpose(
                        pt[:, j * P:(j + 1) * P], srcs[i + j], ident[:]
                    )
                nc.vector.tensor_copy(
                    dst_full[:, i * P:(i + n) * P], pt[:, : n * P]
                )
                i += n

        big2 = ctx.enter_context(tc.tile_pool(name="big2", bufs=1))
        # q (scaled) and k, feature-major, 2-slot rotation keyed by pair%2
        qk = big2.tile([P, 2, 2, NTOK], BF16)
        v_sb = big2.tile([P, TT, H, P], BF16)  # [1 | 63 zeros | v] per head
        ot_all = big2.tile([P, CT, NTOK], BF16)     # attention out, feature-major

        # v_aug layout per head: col 0 = ones (softmax denominator row),
        # cols 1:64 = zeros (padding so O lands at partitions 64:128)
        nc.gpsimd.memset(v_sb[:, :, :, 0:64], 0.0)
        nc.gpsimd.memset(v_sb[:, :, :, 0:1], 1.0)

        # ---- qkv projection pieces (emitted interleaved below) ----
        def emit_qk_chunk(hp, which, c0, cw):
            """qk[slot, which] = (q|k) row block of head pair hp,
            feature-major, for token chunk [c0, c0+cw)."""
            ft = hp + 6 * which
            ps = ps_fill.tile([P, 512], F32, tag="fill", name=f"qkp{ft}_{c0}")
            for ct in range(CT):
                nc.tensor.matmul(
                    ps[:, :cw],
                    wT[:, ct, ft * P:(ft + 1) * P],
                    xT[:, ct, c0:c0 + cw],
                    start=(ct == 0), stop=(ct == CT - 1),
                )
            if which == 0:  # q: fold in softmax scale
                nc.vector.tensor_scalar_mul(
                    qk[:, hp % 2, 0, c0:c0 + cw], ps[:, :cw], SCALE
                )
            else:
                nc.vector.tensor_copy(qk[:, hp % 2, 1, c0:c0 + cw], ps[:, :cw])

        def qk_pair_chunks(p):
            return [(p, w, c0, cw)
                    for c0, cw in ((0, 512), (512, 512), (1024, 256))
                    for w in (0, 1)]

        # v token-major: v[tok, f] = x @ qkv_w.T cols [1536, 2304)
        def emit_v_chunk(tt, half):
            c0, cw, h0, nh = ((0, 512, 0, 8), (512, 256, 8, 4))[half]
            ps = ps_fill.tile([P, 512], F32, tag="fill", name=f"vp{tt}_{half}")
            for ct in range(CT):
                nc.tensor.matmul(
                    ps[:, :cw],
                    xT[:, ct, tt * P:(tt + 1) * P],
                    wT[:, ct, 2 * C + c0:2 * C + c0 + cw],
                    start=(ct == 0), stop=(ct == CT - 1),
                )
            nc.vector.tensor_copy(
                v_sb[:, tt, h0:h0 + nh, 64:128],
                ps[:, :cw].rearrange("p (h e) -> p h e", e=HD),
            )

        # ---- output projection ----
        out_tiles = {}

        def emit_proj_chunk(tt, half):
            c0, cw = ((0, 512), (512, 256))[half]
            if half == 0:
                out_tiles[tt] = outp.tile([P, C], F32, tag="out",
                                          name=f"out{tt}")
            osb = out_tiles[tt]
            ps = ps_fill.tile([P, 512], F32, tag="fill", name=f"prj{tt}_{c0}")
            for ct in range(CT):
                nc.tensor.matmul(
                    ps[:, :cw],
                    ot_all[:, ct, tt * P:(tt + 1) * P],
                    pwT[:, ct, c0:c0 + cw],
                    start=(ct == 0), stop=(ct == CT - 1),
                )
            nc.vector.tensor_tensor(
                osb[:, c0:c0 + cw], ps[:, :cw], bias_bc[:, c0:c0 + cw], ADD,
            )
            if half == 1:
                # split the writeback across two queues to shorten the tail
                t0 = tt * P
                nc.sync.dma_start(out_ext.ap()[t0:t0 + 64, :], osb[0:64, :])
                nc.sync.dma_start(out_ext.ap()[t0 + 64:t0 + P, :],
                                  osb[64:128, :])

        def emit_filler(kind, arg):
            if kind == "qk":
                emit_qk_chunk(*arg)
            elif kind == "v":
                emit_v_chunk(*arg)
            elif kind == "proj":
                emit_proj_chunk(*arg)
            elif kind == "wt":
                # deferred q/k weight row transpose: (g, j) -> wT column
                # blocks (g*6+j)*128 of every channel tile
                g, j = arg
                transpose_to(
                    [wdef_tiles[j][:, g, ct * P:(ct + 1) * P]
                     for ct in range(CT)],
                    [wT[:, ct, (g * 6 + j) * P:(g * 6 + j + 1) * P]
                     for ct in range(CT)],
                )
            elif kind == "wd":
                emit_wd_dma(arg)
            else:  # "pw": deferred proj_w transpose for channel tile arg
                transpose_blocks(
                    [pg[:, j, arg * P:(arg + 1) * P] for j in range(CT)],
                    pwT[:, arg, :],
                )

        # ---- attention helpers ----
        def qh(h, c0, cw):
            b = (h % 2) * 64
            return qk[b:b + 64, (h // 2) % 2, 0, c0:c0 + cw]

        def kh(h, tk):
            b = (h % 2) * 64
            return qk[b:b + 64, (h // 2) % 2, 1, tk * P:(tk + 1) * P]

        def normalize(h, ot_ps, c0, cw):
            """ot_ps: [128, cw] psum (row 0 = denominators, rows 64:128 = O.T
            for tq cols [c0, c0+cw)). Normalize and write to ot_all, fully
            off the ACT queue."""
            b = (h % 2) * 64
            den = dn.tile([P, 512], F32, tag="dn")
            nc.vector.tensor_copy(den[:, :cw], ot_ps[:, :cw])
            rb = rbp.tile([P, 512], F32, tag="rb")
            nc.gpsimd.partition_broadcast(rb[:, :cw], den[0:1, :cw])
            nc.vector.reciprocal_approx_fast(rb[:, :cw], rb[:, :cw])
            nc.vector.tensor_tensor(
                ot_all[b:b + 64, h // 2, c0:c0 + cw],
                den[64:128, :cw], rb[64:128, :cw], MULT,
            )

        def emit_template_scores(hp):
            h0, h1 = 2 * hp, 2 * hp + 1
            st_t = ps_sc.tile([P, 1024], F32, tag="sc", name=f"tst{hp}")
            for tj in range(2):
                for hi, h in enumerate((h0, h1)):
                    nc.tensor.matmul(
                        st_t[:, hi * 512 + tj * NT: hi * 512 + (tj + 1) * NT],
                        kh(h, tj), qh(h, 0, NT), start=True, stop=True,
                    )
            pt_t = pts.tile([P, 1024], BF16, tag="pt", name=f"tpt{hp}")
            nc.scalar.activation(pt_t[:], st_t[:], EXP)
            return pt_t

        def emit_template_av(hp, pt_t):
            for hi, h in enumerate((2 * hp, 2 * hp + 1)):
                to = ps_fill.tile([P, 512], F32, tag="fill", name=f"to{h}")
                for tj in range(2):
                    nc.tensor.matmul(
                        to[:, :NT], v_sb[:, tj, h, :],
                        pt_t[:, hi * 512 + tj * NT: hi * 512 + (tj + 1) * NT],
                        start=(tj == 0), stop=(tj == 1),
                    )
                normalize(h, to, 0, NT)

        # ---- startup: DMA-priority-ordered load + transpose ----
        with tc.tile_pool(name="staging", bufs=2) as staging:
            # x tokens 0-639
            xg0 = staging.tile([P, CT, C], F32, tag="g", name="xg0")
            for j in range(5):
                nc.sync.dma_start(xg0[:, j, :], x_ext.ap()[j * P:(j + 1) * P, :])
            for ct in range(CT):
                transpose_blocks(
                    [xg0[:, j, ct * P:(ct + 1) * P] for j in range(5)],
                    xT[:, ct, 0:640],
                )
            # q0 / k0 weight rows (W rows 0-127 and 768-895)
            wj0 = staging.tile([P, CT, C], F32, tag="g", name="wj0")
            nc.sync.dma_start(wj0[:, 0, :], w_ext.ap()[0:P, :])
            nc.sync.dma_start(wj0[:, 1, :], w_ext.ap()[6 * P:7 * P, :])
            for ct in range(CT):
                transpose_to(
                    [wj0[:, g, ct * P:(ct + 1) * P] for g in range(2)],
                    [wT[:, ct, g * 6 * P:(g * 6 + 1) * P] for g in range(2)],
                )
            # first q/k chunk of pair 0 -> template scores can start
            emit_qk_chunk(0, 0, 0, 512)
            emit_qk_chunk(0, 1, 0, 512)
            pt_t0 = emit_template_scores(0)
            # v weights (W rows 1536-2303)
            wg2 = staging.tile([P, CT, C], F32, tag="g", name="wg2")
            for j in range(CT):
                nc.sync.dma_start(wg2[:, j, :],
                                  w_ext.ap()[(12 + j) * P:(13 + j) * P, :])
            for ct in range(CT):
                transpose_blocks(
                    [wg2[:, j, ct * P:(ct + 1) * P] for j in range(CT)],
                    wT[:, ct, 2 * C:3 * C],
                )
            for tt in (0, 1):
                emit_v_chunk(tt, 0)
                emit_v_chunk(tt, 1)
            emit_template_av(0, pt_t0)
            # x tokens 640-1279
            xg1 = staging.tile([P, CT, C], F32, tag="g", name="xg1")
            for j in range(5):
                t0 = (5 + j) * P
                nc.sync.dma_start(xg1[:, j, :], x_ext.ap()[t0:t0 + P, :])
            for ct in range(CT):
                transpose_blocks(
                    [xg1[:, j, ct * P:(ct + 1) * P] for j in range(5)],
                    xT[:, ct, 640:1280],
                )
            # remaining q/k chunks of pair 0
            for a in qk_pair_chunks(0)[2:]:
                emit_qk_chunk(*a)
            # deferred weight rows: j1 now (needed in pair 0 cj 1); the
            # rest are issued as pure-DMA fillers one pair ahead; proj_w last
            emit_wd_dma(1)
            for j in range(CT):
                nc.sync.dma_start(pg[:, j, :], pw_ext.ap()[j * P:(j + 1) * P, :])

        # ---- attention main loop ----
        for hp in range(6):
            h0, h1 = 2 * hp, 2 * hp + 1
            # filler schedule: deadline for weight row j is pair j's
            # template; v tiles stream JIT inside pair 0 cj 0
            if hp == 0:
                pend = [[("wd", 2)] + [("v", (tt, half)) for tt in range(2, TT)
                                       for half in (0, 1)],
                        [("wd", 3), ("wt", (0, 1)), ("wt", (1, 1))]
                        + [("qk", a) for a in qk_pair_chunks(1)]]
            elif hp == 1:
                nxt = [("qk", a) for a in qk_pair_chunks(2)]
                pend = [[("wd", 4), ("wt", (0, 2)), ("wt", (1, 2))] + nxt[:2],
                        nxt[2:] + [("wt", (0, 3)), ("wt", (1, 3))]]
            elif hp == 2:
                nxt = [("qk", a) for a in qk_pair_chunks(3)]
                pend = [[("wd", 5)] + nxt[:3] + [("wt", (0, 4))],
                        nxt[3:] + [("wt", (1, 4))]]
            elif hp == 3:
                nxt = [("qk", a) for a in qk_pair_chunks(4)]
                pend = [nxt[:3] + [("wt", (0, 5))],
                        nxt[3:] + [("wt", (1, 5))]]
            elif hp == 4:
                pend = [[("qk", a) for a in qk_pair_chunks(5)],
                        [("pw", ct) for ct in range(CT)]]
            else:
                pend = [[], []]  # cj1 list filled after cj0 normalize

            if 1 <= hp <= 4:
                pt_t = emit_template_scores(hp)
                emit_template_av(hp, pt_t)
            # (pair 0's template ran in startup; pair 5's runs after its
            # search so the tail projections can overlap it)

            for cj in range(2):
                c0 = NT + cj * 512
                pending = pend[cj]
                ots = {h: ps_ot.tile([P, 512], F32, tag="ot",
                                     name=f"ot{h}_{cj}")
                       for h in (h0, h1)}
                for tk in range(TT):
                    st = ps_sc.tile([P, 1024], F32, tag="sc",
                                    name=f"st{hp}_{cj}_{tk}")
                    for hi, h in enumerate((h0, h1)):
                        nc.tensor.matmul(
                            st[:, hi * 512:(hi + 1) * 512],
                            kh(h, tk), qh(h, c0, 512), start=True, stop=True,
                        )
                    pt = pts.tile([P, 1024], BF16, tag="pt",
                                  name=f"pt{hp}_{cj}_{tk}")
                    nc.scalar.activation(pt[:], st[:], EXP)
                    for hi, h in enumerate((h0, h1)):
                        nc.tensor.matmul(
                            ots[h][:, :], v_sb[:, tk, h, :],
                            pt[:, hi * 512:(hi + 1) * 512],
                            start=(tk == 0), stop=(tk == TT - 1),
                        )
                    for _ in range(2 if (hp == 0 and cj == 0) else 1):
                        if pending:
                            emit_filler(*pending.pop(0))
                for h in (h0, h1):
                    normalize(h, ots[h], c0, 512)
                while pending:
                    emit_filler(*pending.pop(0))
                if hp == 5 and cj == 0:
                    # queries 256-767 (tiles 2-5) are final once every pair's
                    # cj-0 normalize is done -> proj them during cj 1
                    pend[1] = [("proj", (tt, half)) for tt in (2, 3, 4, 5)
                               for half in (0, 1)]

        # tail: pair-5 template runs now; proj tiles 6-9 (final after its
        # cj-1 normalize) overlap the template's exp/AV, then tiles 0-1
        pt_t5 = emit_template_scores(5)
        for tt in (6, 7, 8, 9):
            for half in (0, 1):
                emit_proj_chunk(tt, half)
        emit_template_av(5, pt_t5)
        for tt in (0, 1):
            for half in (0, 1):
                emit_proj_chunk(tt, half)

    nc.compile()
    return nc


_NC = None


def _get_nc():
    global _NC
    if _NC is None:
        _NC = build_nc()
    return _NC


def kernel(x, qkv_w, proj_w, proj_b, **_ignored):
    from concourse.bass_utils import run_bass_kernel_spmd

    x = np.ascontiguousarray(np.asarray(x), dtype=np.float32)
    qkv_w = np.ascontiguousarray(np.asarray(qkv_w), dtype=np.float32)
    proj_w = np.ascontiguousarray(np.asarray(proj_w), dtype=np.float32)
    proj_b = np.ascontiguousarray(np.asarray(proj_b), dtype=np.float32).reshape(1, C)

    nc = _get_nc()
    in_maps = [
        {"x": x[i], "qkv_w": qkv_w, "proj_w": proj_w, "proj_b": proj_b}
        for i in range(8)
    ]
    res = run_bass_kernel_spmd(nc, in_maps, list(range(8)))
    return np.stack([res.results[i]["out"] for i in range(8)])


if __name__ == "__main__":
    rng = np.random.default_rng(0)
    ins = {
        "x": rng.standard_normal((8, NTOK, C), dtype=np.float32),
        "qkv_w": rng.standard_normal((3 * C, C), dtype=np.float32) * 0.02,
        "proj_w": rng.standard_normal((C, C), dtype=np.float32) * 0.02,
        "proj_b": np.zeros(C, dtype=np.float32),
    }
    out = kernel(**ins)
    print("out", out.shape, out.dtype)
